# revision 10
# baseline (speedup 1.0000x reference)
"""Triplet-margin loss (EuclideanTriple) on 8 Trainium2 NeuronCores.

loss = sum_i relu( ||x_i - y_i + eps||_2 + margin - ||x_i - z_i + eps||_2 )

Data-parallel: N=131072 rows sharded 8 ways (16384 rows/core). Each core
computes its partial hinge sums reduced to a [128,1] per-partition vector;
the host sums the 8x128 partials into the final scalar.

Per-core layout: rows -> partitions. Chunks of 2048 rows are loaded as
[128, 16*256] tiles (16 consecutive rows per partition -> 16 KiB contiguous
per-partition DMA reads). Pipeline per chunk:
  DVE : u = (x + eps) - y   (scalar_tensor_tensor, in-place into y/z tile)
  ACT : u = u^2             (Square)
  DVE : dsq[:, chunk] = reduce_sum over D ([128,16,256] -> [128,16])
Tail (once): ACT sqrt over both accumulated dsq halves, DVE hinge combine,
ACT relu with accum_out -> per-partition sums, DMA out.
"""

from contextlib import ExitStack

import numpy as np

import concourse.bacc as bacc
import concourse.bass as bass
import concourse.mybir as mybir
import concourse.tile as tile
from concourse import bass_utils

N_TOTAL = 131072
D = 256
N_CORES = 8
SHARD = N_TOTAL // N_CORES  # 16384 rows per core
P = 128                     # SBUF partitions
RPP = SHARD // P            # 128 rows per partition (whole shard)
CHUNK_A = 16                # rows per partition per chunk
N_CHUNKS = RPP // CHUNK_A   # 8 chunks
FD = CHUNK_A * D            # 4096 free-dim elements per chunk tile
MARGIN = 0.5
EPS = 1e-6
F32 = mybir.dt.float32
IO_BUFS = 2


def build_nc(repeat: int = 1) -> bass.Bass:
    nc = bacc.Bacc("TRN2", target_bir_lowering=False, debug=False)
    x = nc.dram_tensor("x", [SHARD, D], F32, kind="ExternalInput").ap()
    y = nc.dram_tensor("y", [SHARD, D], F32, kind="ExternalInput").ap()
    z = nc.dram_tensor("z", [SHARD, D], F32, kind="ExternalInput").ap()
    out = nc.dram_tensor("out", [P, 1], F32, kind="ExternalOutput").ap()

    act = mybir.ActivationFunctionType

    with tile.TileContext(nc) as tc:
        with ExitStack() as ctx:
            io = ctx.enter_context(tc.tile_pool(name="io", bufs=IO_BUFS))
            acc = ctx.enter_context(tc.tile_pool(name="acc", bufs=1))

            # per-row squared distances: cols [0:RPP] = pos pair, [RPP:2*RPP] = neg
            dsq = acc.tile([P, 2 * RPP], F32, tag="dsq")

            # const bias vectors for ACT (bias must be an AP)
            eps_t = acc.tile([P, 1], F32, tag="eps")
            nc.vector.memset(eps_t[:], EPS)
            mar_t = acc.tile([P, 1], F32, tag="mar")
            nc.vector.memset(mar_t[:], MARGIN)

            for _ in range(repeat):
                for c in range(N_CHUNKS):
                    rows = slice(c * P * CHUNK_A, (c + 1) * P * CHUNK_A)
                    xt = io.tile([P, FD], F32, tag="xt")
                    yt = io.tile([P, FD], F32, tag="yt")
                    zt = io.tile([P, FD], F32, tag="zt")
                    nc.sync.dma_start(
                        xt[:], x[rows, :].rearrange("(p a) d -> p (a d)", p=P)
                    )
                    nc.sync.dma_start(
                        yt[:], y[rows, :].rearrange("(p a) d -> p (a d)", p=P)
                    )
                    nc.sync.dma_start(
                        zt[:], z[rows, :].rearrange("(p a) d -> p (a d)", p=P)
                    )
                    # u = x - y in place into the y/z tiles, then (u + eps)^2
                    # on ACT (the +eps rides ACT's free bias)
                    nc.vector.tensor_sub(yt[:], xt[:], yt[:])
                    nc.vector.tensor_sub(zt[:], xt[:], zt[:])
                    nc.scalar.activation(yt[:], yt[:], act.Square, bias=eps_t[:])
                    nc.scalar.activation(zt[:], zt[:], act.Square, bias=eps_t[:])
                    cols = slice(c * CHUNK_A, (c + 1) * CHUNK_A)
                    nc.vector.reduce_sum(
                        dsq[:, cols],
                        yt[:].rearrange("p (a d) -> p a d", a=CHUNK_A),
                        axis=mybir.AxisListType.X,
                    )
                    nc.vector.reduce_sum(
                        dsq[:, RPP + c * CHUNK_A : RPP + (c + 1) * CHUNK_A],
                        zt[:].rearrange("p (a d) -> p a d", a=CHUNK_A),
                        axis=mybir.AxisListType.X,
                    )

                # tail: dists, hinge (margin rides Relu's bias), per-partition sum
                droot = acc.tile([P, 2 * RPP], F32, tag="droot")
                nc.scalar.activation(droot[:], dsq[:], act.Sqrt)
                hing = acc.tile([P, RPP], F32, tag="hing")
                nc.vector.tensor_sub(hing[:], droot[:, :RPP], droot[:, RPP:])
                relu_t = acc.tile([P, RPP], F32, tag="relu")
                hsum = acc.tile([P, 1], F32, tag="hsum")
                nc.scalar.activation(
                    relu_t[:], hing[:], act.Relu, bias=mar_t[:], accum_out=hsum[:]
                )
                nc.sync.dma_start(out[:], hsum[:])
    nc.compile()
    return nc


def _run(nc: bass.Bass, x, y, z):
    in_maps = [
        {
            "x": np.ascontiguousarray(x[i * SHARD : (i + 1) * SHARD]),
            "y": np.ascontiguousarray(y[i * SHARD : (i + 1) * SHARD]),
            "z": np.ascontiguousarray(z[i * SHARD : (i + 1) * SHARD]),
        }
        for i in range(N_CORES)
    ]
    return bass_utils.run_bass_kernel_spmd(
        nc, in_maps, core_ids=list(range(N_CORES))
    )


def kernel(x: np.ndarray, y: np.ndarray, z: np.ndarray) -> np.ndarray:
    x = np.asarray(x, dtype=np.float32)
    y = np.asarray(y, dtype=np.float32)
    z = np.asarray(z, dtype=np.float32)
    res = _run(build_nc(1), x, y, z)
    total = np.float64(0.0)
    for r in res.results:
        total += r["out"].astype(np.float64).sum()
    return np.float32(total)
